# revision 55
# baseline (speedup 1.0000x reference)
"""Causal multi-head self-attention on 8 Trainium2 NeuronCores (Bass/Tile).

Problem (hardcoded): x [4, 2048, 1024] fp32, W_qkv [1024, 3072], b_qkv [3072],
W_out [1024, 1024], b_out [1024]. 16 heads, head_dim 64.

Sharding: core c = 2*b + g handles batch b (4 batches) and head group g
(8 heads): tensor-parallel over heads within a batch pair. Each core computes
qkv projection for its 8 heads, causal flash attention, and a partial output
projection (its 512 rows of W_out). The two partials per batch are summed on
the host (the "all-reduce") along with b_out.

Design notes (v2):
 - all matmul operands are bf16 (1 cyc/row on the PE at any moving width;
   fp32r would drop to 4 cyc/row for the narrow diagonal tiles). PSUM
   accumulation stays fp32, so only input quantization (~0.4%) is lost.
 - scores^T tiles [kj, qi] as in v1 (z-skip of fully-masked 128-col groups,
   -1e6 causal bias added via identity x mask matmul on the diagonal tiles,
   exp without max-subtraction on ACT, output bf16).
 - AV uses the ex tile as the *stationary* and v [128kj, 65] as the moving:
   cost is 65 rows per (head, qtile, kj-tile) instead of 128 — about half
   the moving rows of the v1 orientation. Output lands naturally as
   [q, hd] with the appended ones-column giving the softmax denominator in
   column 64, so normalization is a per-partition reciprocal + broadcast
   multiply on DVE (no PE broadcast matmuls at all).
 - normalized attention tiles are transposed back to [hd, q] through the PE
   (pure transpose against a bf16 identity, 128 rows each) to feed the
   output projection, which is unchanged (Wo stationary, attn^T moving).
 - k/v projections are emitted just-in-time inside each block's attention
   (the diagonal kj tiles are the last consumers), giving the PE fill work
   during the ACT-paced exp stretches of late blocks; q projections stay
   ahead of their block. Out-projection of block qb-1 is emitted after
   attention(qb) as lower-priority fill (v1 pattern).
 - yt evictions run on the otherwise-idle Pool engine; output DMAs go on
   the SP queue so the ACT sequencer never stalls on DMA config.
"""
import numpy as np

import concourse.bacc as bacc
import concourse.tile as tile
from concourse import mybir
from concourse.bass import broadcast_tensor_aps
from concourse.bass_utils import run_bass_kernel_spmd

B, L, D = 4, 2048, 1024
NH, HD = 16, 64
G = 8            # heads per core
NP = G // 2      # head pairs per core
LC = 512         # qi block
KT = 128         # kj tile
NKJ = L // KT    # 16
NLC = L // LC    # 4 qi blocks
CH = 256         # qkv l-chunk
NKT = D // 128   # 8 contraction tiles
F32 = mybir.dt.float32
BF = mybir.dt.bfloat16
AF = mybir.ActivationFunctionType

_cache = {}


def _build(trace_names=False):
    nc = bacc.Bacc("TRN2", target_bir_lowering=False, debug=False, num_devices=8)
    xT = nc.dram_tensor("xT", [NKT, 128, L], BF, kind="ExternalInput")
    W_in = nc.dram_tensor("W_in", [NKT, 128, 3 * G * HD], BF,
                          kind="ExternalInput")
    W_out_s = nc.dram_tensor("W_out_s", [NP, 128, D], BF, kind="ExternalInput")
    masks = nc.dram_tensor("masks", [128, 128], BF, kind="ExternalInput")
    ident = nc.dram_tensor("ident", [128, 128], BF, kind="ExternalInput")
    yT = nc.dram_tensor("yT", [D, L], F32, kind="ExternalOutput")

    scale = float(1.0 / np.sqrt(HD))
    WG = 256                  # W dma col-group width
    NWG = (3 * G * HD) // WG  # 6 groups

    with tile.TileContext(nc) as tc:
        with tc.tile_pool(name="store", bufs=1) as store, \
             tc.tile_pool(name="qtp", bufs=2) as qtp, \
             tc.tile_pool(name="xtp", bufs=8) as xtp, \
             tc.tile_pool(name="expp", bufs=34) as expp, \
             tc.tile_pool(name="attnp", bufs=2) as attnp, \
             tc.tile_pool(name="atp", bufs=4) as atp, \
             tc.tile_pool(name="denp", bufs=2) as denp, \
             tc.tile_pool(name="ytp", bufs=3) as ytp, \
             tc.tile_pool(name="mm_ps", bufs=4, space="PSUM") as mm_ps, \
             tc.tile_pool(name="scores", bufs=2, space="PSUM") as scores_p:
            W_sb = store.tile([128, NKT, 3 * G * HD], BF)
            Wo_sb = store.tile([128, NP, D], BF)
            kT_sb = store.tile([128, NP, L], BF)
            v_sb = store.tile([128, NKJ, G, HD + 1], BF)
            masks_sb = store.tile([128, 128], BF)
            id_sb = store.tile([128, 128], BF)

            nc.vector.memset(v_sb[:, :, :, HD:HD + 1], 1.0)

            xT_r = xT.rearrange("kt p l -> p kt l")
            W_r = W_in.rearrange("kt p c -> p kt c")
            yT_r = yT.rearrange("(m p) l -> p m l", p=128)

            # DMA order = first-use order: xt0, W[q m01], W[k m01],
            # ident, xt1, masks, W[q m23], W[k m23], W[v], Wo.  Startup
            # loads go on the scalar queue (ACT idle then); mid-kernel x
            # chunks and yt outputs go on the SP queue so the ACT
            # sequencer never stalls on DMA config mid-exp-stream.
            xt_pre = [xtp.tile([128, NKT, CH], BF, name=f"xt{c}", tag="xt")
                      for c in range(2)]

            def wdma(g):
                nc.scalar.dma_start(out=W_sb[:, :, g * WG:(g + 1) * WG],
                                    in_=W_r[:, :, g * WG:(g + 1) * WG])

            nc.sync.dma_start(out=xt_pre[0][:], in_=xT_r[:, :, 0:CH])
            wdma(0)
            nc.gpsimd.dma_start(out=id_sb[:], in_=ident[:])
            nc.gpsimd.dma_start(out=masks_sb[:], in_=masks[:])
            wdma(2)
            nc.sync.dma_start(out=xt_pre[1][:],
                               in_=xT_r[:, :, CH:2 * CH])
            wdma(1)
            wdma(3)
            wdma(4)
            wdma(5)
            nc.scalar.dma_start(
                out=Wo_sb[:], in_=W_out_s.rearrange("ct p d -> p ct d"))

            # p-state warmup: keep the PE continuously busy from t~0 so
            # it reaches full clock before the first real matmul's inputs
            # arrive (the first ~5us are DMA-bound anyway)
            warm = store.tile([128, 128], BF)
            nc.vector.memset(warm[:], 0.25)
            wps = mm_ps.tile([128, 128], F32, tag="ps", name="wps")
            for _ in range(48):
                nc.tensor.matmul(wps[:], warm[:], warm[:], start=True,
                                 stop=True)

            xts = {}

            def get_xt(c):
                if c not in xts:
                    if c < 2:
                        xts[c] = xt_pre[c]
                    else:
                        xt = xtp.tile([128, NKT, CH], BF, name=f"xt{c}",
                                      tag="xt")
                        nc.sync.dma_start(out=xt[:],
                                          in_=xT_r[:, :, c * CH:(c + 1) * CH])
                        xts[c] = xt
                return xts[c]

            KOFF = G * HD
            VOFF = 2 * G * HD

            def qk_m(c, m, qT_blk):
                """q and k projections for one m (head-pair) tile of one
                x chunk — the minimal unit on the first block's critical
                path."""
                xt = get_xt(c)
                half = (c % 2) * CH
                for off, out_ap in (
                        (m * 128, qT_blk[:, m, half:half + CH]),
                        (KOFF + m * 128, kT_sb[:, m, c * CH:c * CH + CH])):
                    ps = mm_ps.tile([128, CH], F32, tag="ps", name="psqk")
                    for kt in range(NKT):
                        nc.tensor.matmul(
                            ps[:], W_sb[:, kt, off:off + 128],
                            xt[:, kt, :], start=(kt == 0), stop=(kt == NKT - 1))
                    nc.vector.tensor_copy(out=out_ap, in_=ps[:])

            def v_proj(c):
                xt = get_xt(c)
                for sub in range(CH // KT):
                    ps = mm_ps.tile([128, G * HD], F32, tag="ps", name="psv")
                    for kt in range(NKT):
                        nc.tensor.matmul(
                            ps[:], xt[:, kt, sub * KT:(sub + 1) * KT],
                            W_sb[:, kt, VOFF:VOFF + G * HD],
                            start=(kt == 0), stop=(kt == NKT - 1))
                    nc.vector.tensor_copy(
                        out=v_sb[:, c * (CH // KT) + sub, :, 0:HD],
                        in_=ps[:].rearrange("p (h d) -> p h d", h=G))

            def q_parts(c, m, qT_blk):
                """Two ~426ns queue items sharing one open PSUM group, so
                a ready score matmul never waits behind a long fill item."""
                st = {}
                half = (c % 2) * CH

                def p0():
                    xt = get_xt(c)
                    st["ps"] = mm_ps.tile([128, CH], F32, tag="ps", name="psq")
                    for kt in range(4):
                        nc.tensor.matmul(
                            st["ps"][:], W_sb[:, kt, m * 128:(m + 1) * 128],
                            xt[:, kt, :], start=(kt == 0), stop=False)

                def p1():
                    xt = get_xt(c)
                    for kt in range(4, NKT):
                        nc.tensor.matmul(
                            st["ps"][:], W_sb[:, kt, m * 128:(m + 1) * 128],
                            xt[:, kt, :], start=False, stop=(kt == NKT - 1))
                    nc.vector.tensor_copy(out=qT_blk[:, m, half:half + CH],
                                          in_=st["ps"][:])
                return [(CH, p0), (CH, p1)]

            def k_parts(c, m):
                st = {}

                def p0():
                    xt = get_xt(c)
                    st["ps"] = mm_ps.tile([128, CH], F32, tag="ps", name="psk")
                    for kt in range(4):
                        nc.tensor.matmul(
                            st["ps"][:],
                            W_sb[:, kt, KOFF + m * 128:KOFF + (m + 1) * 128],
                            xt[:, kt, :], start=(kt == 0), stop=False)

                def p1():
                    xt = get_xt(c)
                    for kt in range(4, NKT):
                        nc.tensor.matmul(
                            st["ps"][:],
                            W_sb[:, kt, KOFF + m * 128:KOFF + (m + 1) * 128],
                            xt[:, kt, :], start=False, stop=(kt == NKT - 1))
                    nc.vector.tensor_copy(
                        out=kT_sb[:, m, c * CH:c * CH + CH], in_=st["ps"][:])
                return [(CH, p0), (CH, p1)]

            def v_parts(c, sub):
                st = {}

                def mk(kt0, kt1, first, last):
                    def p():
                        xt = get_xt(c)
                        if first:
                            st["ps"] = mm_ps.tile([128, G * HD], F32,
                                                  tag="ps", name="psv")
                        for kt in range(kt0, kt1):
                            nc.tensor.matmul(
                                st["ps"][:], xt[:, kt, sub * KT:(sub + 1) * KT],
                                W_sb[:, kt, VOFF:VOFF + G * HD],
                                start=(kt == 0), stop=(kt == NKT - 1))
                        if last:
                            nc.vector.tensor_copy(
                                out=v_sb[:, c * (CH // KT) + sub, :, 0:HD],
                                in_=st["ps"][:].rearrange("p (h d) -> p h d",
                                                          h=G))
                    return p
                return [(G * HD, mk(0, 2, True, False)),
                        (G * HD, mk(2, 4, False, False)),
                        (G * HD, mk(4, 6, False, False)),
                        (G * HD, mk(6, 8, False, True))]

            attn_nats = {}

            def av_seg(qb, pair, j, exs, st, t0, t1):
                """AV accumulation segment [t0, t1) for one qtile of a
                pair; the last segment normalizes out of PSUM."""
                hA, hB = 2 * pair, 2 * pair + 1
                jt = qb * (LC // KT) + j
                if t0 == 0:
                    if j == 0:
                        attn_nats[pair] = attnp.tile([128, 4, 128], BF,
                                                     name="anat")
                    st["avA"] = mm_ps.tile([128, LC], F32, tag="ps",
                                           name="avA")
                    st["avB"] = mm_ps.tile([128, LC], F32, tag="ps",
                                           name="avB")
                avA, avB = st["avA"], st["avB"]
                for t in range(t0, min(t1, jt + 1)):
                    nc.tensor.matmul(
                        avA[:, 0:HD + 1], exs[t][:, j * KT:(j + 1) * KT],
                        v_sb[:, t, hA, :], start=(t == 0), stop=(t == jt))
                    nc.tensor.matmul(
                        avB[:, 0:HD + 1],
                        exs[t][:, LC + j * KT:LC + (j + 1) * KT],
                        v_sb[:, t, hB, :], start=(t == 0), stop=(t == jt))
                if t1 < jt + 1:
                    return
                attn_nat = attn_nats[pair]
                den = denp.tile([128, 2], F32, name="den")
                for h2, av in ((0, avA), (1, avB)):
                    nc.vector.reciprocal(out=den[:, h2:h2 + 1],
                                         in_=av[:, HD:HD + 1])
                    nc.vector.tensor_scalar(
                        out=attn_nat[:, j, h2 * HD:(h2 + 1) * HD],
                        in0=av[:, 0:HD], scalar1=den[:, h2:h2 + 1],
                        scalar2=None, op0=mybir.AluOpType.mult)

            def av_group(qb, pair, j, exs):
                st = {}
                av_seg(qb, pair, j, exs, st, 0, 99)

            def transpose_j(pair, j, attnT):
                attn_nat = attn_nats[pair]
                tp = mm_ps.tile([128, KT], BF, tag="ps", name="tp")
                nc.tensor.matmul(tp[:], attn_nat[:, j, :], id_sb[:],
                                 is_transpose=True)
                nc.vector.tensor_copy(
                    out=attnT[:, pair, j * KT:(j + 1) * KT], in_=tp[:])

            def transposes(pair, attnT):
                for j in range(4):
                    transpose_j(pair, j, attnT)
                attn_nats.pop(pair)

            def outproj_p0(qb, m, attnT, st):
                st["ps"] = mm_ps.tile([128, LC], F32, tag="ps", name="psy")
                for ct in range(2):
                    nc.tensor.matmul(
                        st["ps"][:], Wo_sb[:, ct, m * 128:(m + 1) * 128],
                        attnT[:, ct, :], start=(ct == 0), stop=False)

            def outproj_p1(qb, m, attnT, st):
                for ct in range(2, NP):
                    nc.tensor.matmul(
                        st["ps"][:], Wo_sb[:, ct, m * 128:(m + 1) * 128],
                        attnT[:, ct, :], start=False, stop=(ct == NP - 1))
                yt = ytp.tile([128, LC], F32, name="yt", tag="yt")
                nc.vector.tensor_copy(out=yt[:], in_=st["ps"][:])
                nc.sync.dma_start(out=yT_r[:, m, qb * LC:qb * LC + LC],
                                  in_=yt[:])

            yt_last = {}

            def outproj_mj(m, j, attnT, yts):
                ps = mm_ps.tile([128, KT], F32, tag="ps", name="psj")
                for ct in range(NP):
                    nc.tensor.matmul(
                        ps[:], Wo_sb[:, ct, m * 128:(m + 1) * 128],
                        attnT[:, ct, j * KT:(j + 1) * KT],
                        start=(ct == 0), stop=(ct == NP - 1))
                nc.vector.tensor_copy(out=yts[m][:, j * KT:(j + 1) * KT],
                                      in_=ps[:])

            def outproj_m(qb, m, attnT):
                ps = mm_ps.tile([128, LC], F32, tag="ps", name="psy")
                for ct in range(NP):
                    nc.tensor.matmul(
                        ps[:], Wo_sb[:, ct, m * 128:(m + 1) * 128],
                        attnT[:, ct, :], start=(ct == 0), stop=(ct == NP - 1))
                yt = ytp.tile([128, LC], F32, name="yt", tag="yt")
                nc.vector.tensor_copy(out=yt[:], in_=ps[:])
                nc.sync.dma_start(out=yT_r[:, m, qb * LC:qb * LC + LC],
                                  in_=yt[:])

            # Global fill queue: every PE task that is not on the ACT
            # critical path (scores+exp) is queued with a row-cost and
            # drained into the t-loops at the rate the exp stream frees PE
            # cycles (exp runs at 0.833 ns/col on ACT vs 0.4167 ns/row on
            # PE, so each exp column buys about one spare PE row beyond
            # the score matmuls).  Unspent inventory floats forward into
            # the ACT-heavy late blocks; dependency-forced items are
            # drained explicitly at pair/block boundaries.
            fillq = []
            opq = []   # out-projections: lowest priority, float late
            cur_qb = [0]

            def qfill(rows, fn, cls="", min_qb=-1):
                fns = fn if isinstance(fn, list) else [fn]
                fillq.append([rows, fns, cls, min_qb])

            def _pick(q, cls):
                for i, (rows, fns, c, mq) in enumerate(q):
                    if cls is not None:
                        if c == cls:
                            return i
                        continue
                    if mq <= cur_qb[0]:
                        return i
                return None

            def drain(budget=None, cls=None):
                while True:
                    if budget is not None and budget <= 0:
                        break
                    i = _pick(fillq, cls)
                    if i is not None:
                        rows, fns, c, mq = fillq.pop(i)
                    elif cls is None:
                        j = _pick(opq, None)
                        if j is None:
                            break
                        rows, fns, c, mq = opq.pop(j)
                    else:
                        break
                    for fn in fns:
                        fn()
                    if budget is not None:
                        budget -= rows

            def queue_proj(c, qT_blk):
                for m in range(NP):
                    parts = q_parts(c, m, qT_blk)
                    qfill(sum(r for r, _ in parts), [f for _, f in parts],
                          f"q{c}")
                for m in range(NP):
                    parts = k_parts(c, m)
                    qfill(sum(r for r, _ in parts), [f for _, f in parts],
                          f"kv{c}", c // 2)
                for sub in range(CH // KT):
                    parts = v_parts(c, sub)
                    qfill(sum(r for r, _ in parts), [f for _, f in parts],
                          f"kv{c}", c // 2)

            def queue_av(qb, pair, exs, attnT):
                for j in range(4):
                    jt = qb * (LC // KT) + j
                    qfill(2 * (HD + 1) * (jt + 1),
                          lambda j=j: av_group(qb, pair, j, exs),
                          f"av{pair}")
                for j in range(4):
                    qfill(KT, lambda j=j: transpose_j(pair, j, attnT),
                          f"av{pair}")
                qfill(0, lambda: attn_nats.pop(pair), f"av{pair}")

            def attention(qb, qT_blk, attnT, qT_next):
                n_t = (qb + 1) * (LC // KT)
                last = qb == NLC - 1
                if qT_next is not None:
                    queue_proj(2 * qb + 2, qT_next)
                    qfill(0, lambda: None)
                    queue_proj(2 * qb + 3, qT_next)
                if qb == 0:
                    # pair-0 critical pieces first, then the rest of
                    # chunk 0 (ready as soon as xt0 lands — fills the
                    # PE while xt1 is still in flight), then chunk 1
                    qk_m(0, 0, qT_blk)
                    qk_m(1, 0, qT_blk)
                    for m in range(1, NP):
                        qk_m(0, m, qT_blk)
                    for m in range(1, NP):
                        qk_m(1, m, qT_blk)
                for pair in range(NP):
                    exs = []
                    pace_own_av = last and pair == NP - 1
                    for t in range(n_t):
                        diag = t >= qb * (LC // KT)
                        o = t - qb * (LC // KT) if diag else 0
                        z = o * KT
                        if diag and o == 0 and qb > 0:
                            drain(cls=f"kv{2 * qb}")
                        if diag and o == 2 and qb > 0:
                            drain(cls=f"kv{2 * qb + 1}")
                        sc = scores_p.tile([128, 2 * LC], F32, tag="sc")
                        nc.tensor.matmul(
                            sc[:, z:LC],
                            kT_sb[0:64, pair, t * KT:(t + 1) * KT],
                            qT_blk[0:64, pair, z:LC], start=True,
                            stop=not diag)
                        nc.tensor.matmul(
                            sc[:, LC + z:2 * LC],
                            kT_sb[64:128, pair, t * KT:(t + 1) * KT],
                            qT_blk[64:128, pair, z:LC], start=True,
                            stop=not diag)
                        if diag:
                            nc.tensor.matmul(sc[:, z:z + KT], id_sb[:],
                                             masks_sb[:],
                                             start=False, stop=True)
                            nc.tensor.matmul(sc[:, LC + z:LC + z + KT],
                                             id_sb[:], masks_sb[:],
                                             start=False, stop=True)
                        ex = expp.tile([128, 2 * LC], BF)
                        sc_v = sc[:].rearrange("p (h c) -> p h c", h=2)[:, :, z:LC]
                        ex_v = ex[:].rearrange("p (h c) -> p h c", h=2)[:, :, z:LC]
                        nc.scalar.activation(ex_v, sc_v, AF.Exp, scale=scale)
                        exs.append(ex)
                        if pace_own_av and diag:
                            av_group(qb, pair, o, exs)
                        spare = 2 * (LC - z) - (256 if diag else 0)
                        drain(budget=int(spare * 0.75))
                    # dependency-forced drains: the pair-before-last's AV
                    # must complete (exp-tile pool bound) ...
                    if pair >= 1:
                        drain(cls=f"av{pair - 1}")
                    if pace_own_av:
                        transposes(pair, attnT)
                    else:
                        queue_av(qb, pair, exs, attnT)
                # ... and the next block's q projections before its
                # scores (k/v only feed its diagonal tiles: drained there)
                if qT_next is not None:
                    drain(cls=f"q{2 * qb + 2}")
                    drain(cls=f"q{2 * qb + 3}")
                if not last:
                    drain(cls=f"av{NP - 1}")

            attnTs = {}
            qTs = {0: qtp.tile([128, NP, LC], BF, name="qT0", tag="qT")}
            for qb in range(NLC):
                cur_qb[0] = qb
                if qb + 1 < NLC:
                    qTs[qb + 1] = qtp.tile([128, NP, LC], BF,
                                           name=f"qT{qb + 1}", tag="qT")
                attnTs[qb] = atp.tile([128, NP, LC], BF, name=f"aT{qb}",
                                      tag="aT")
                if qb == 0:
                    for c in range(2):
                        for sub in range(2):
                            parts = v_parts(c, sub)
                            qfill(sum(r for r, _ in parts),
                                  [f for _, f in parts], f"kv{c}")
                attention(qb, qTs[qb], attnTs[qb], qTs.get(qb + 1))
                if qb < NLC - 1:
                    for m in range(D // 128):
                        opq.append([NP * LC,
                                    [lambda m=m, qb=qb:
                                     outproj_m(qb, m, attnTs[qb])], "op",
                                    NLC - 1])
            cur_qb[0] = NLC
            drain()
            for m in range(D // 128):
                outproj_m(NLC - 1, m, attnTs[NLC - 1])
    nc.compile()
    return nc


def _make_masks():
    import ml_dtypes
    r = np.arange(128)[:, None]
    c = np.arange(128)[None, :]
    return np.where(c >= r, 0.0, -1e6).astype(ml_dtypes.bfloat16)


def _make_ident():
    import ml_dtypes
    return np.eye(128, dtype=ml_dtypes.bfloat16)


def kernel(x, W_qkv, b_qkv, W_out, b_out, _trace=False, _trace_kwargs=None):
    import ml_dtypes
    bf16 = ml_dtypes.bfloat16
    x = np.ascontiguousarray(x, dtype=np.float32)
    W_qkv = np.asarray(W_qkv, dtype=np.float32)
    b_qkv = np.asarray(b_qkv, dtype=np.float32)
    W_out = np.asarray(W_out, dtype=np.float32)
    b_out = np.asarray(b_out, dtype=np.float32)
    assert np.all(b_qkv == 0.0), "nonzero b_qkv not supported by this kernel"

    if "nc" not in _cache:
        _cache["nc"] = _build()
    nc = _cache["nc"]

    masks = _make_masks()
    ident = _make_ident()
    Wq, Wk, Wv = W_qkv[:, 0:D], W_qkv[:, D:2 * D], W_qkv[:, 2 * D:3 * D]

    in_maps = []
    for c in range(8):
        b, g = divmod(c, 2)
        cols = slice(g * G * HD, (g + 1) * G * HD)
        W_in = np.concatenate([Wq[:, cols], Wk[:, cols], Wv[:, cols]], axis=1)
        in_maps.append({
            "xT": np.ascontiguousarray(x[b].T).astype(bf16).reshape(
                NKT, 128, L),
            "W_in": W_in.astype(bf16).reshape(NKT, 128, 3 * G * HD),
            "W_out_s": W_out[cols, :].astype(bf16).reshape(NP, 128, D),
            "masks": masks,
            "ident": ident,
        })

    kw = {}
    if _trace:
        kw["trace"] = True
        kw.update(_trace_kwargs or {})
    res = run_bass_kernel_spmd(nc, in_maps, list(range(8)), **kw)

    out = np.empty((B, L, D), dtype=np.float32)
    for b in range(B):
        yT = res.results[2 * b]["yT"] + res.results[2 * b + 1]["yT"]
        out[b] = yT.T + b_out
    if _trace:
        _cache["last_result"] = res
    return out


# revision 58
# speedup vs baseline: 1.0007x; 1.0007x over previous
"""Causal multi-head self-attention on 8 Trainium2 NeuronCores (Bass/Tile).

Problem (hardcoded): x [4, 2048, 1024] fp32, W_qkv [1024, 3072], b_qkv [3072],
W_out [1024, 1024], b_out [1024]. 16 heads, head_dim 64.

Sharding: core c = 2*b + g handles batch b (4 batches) and head group g
(8 heads): tensor-parallel over heads within a batch pair. Each core computes
qkv projection for its 8 heads, causal flash attention, and a partial output
projection (its 512 rows of W_out). The two partials per batch are summed on
the host (the "all-reduce") along with b_out.

Design notes (v2):
 - all matmul operands are bf16 (1 cyc/row on the PE at any moving width;
   fp32r would drop to 4 cyc/row for the narrow diagonal tiles). PSUM
   accumulation stays fp32, so only input quantization (~0.4%) is lost.
 - scores^T tiles [kj, qi] as in v1 (z-skip of fully-masked 128-col groups,
   -1e6 causal bias added via identity x mask matmul on the diagonal tiles,
   exp without max-subtraction on ACT, output bf16).
 - AV uses the ex tile as the *stationary* and v [128kj, 65] as the moving:
   cost is 65 rows per (head, qtile, kj-tile) instead of 128 — about half
   the moving rows of the v1 orientation. Output lands naturally as
   [q, hd] with the appended ones-column giving the softmax denominator in
   column 64, so normalization is a per-partition reciprocal + broadcast
   multiply on DVE (no PE broadcast matmuls at all).
 - normalized attention tiles are transposed back to [hd, q] through the PE
   (pure transpose against a bf16 identity, 128 rows each) to feed the
   output projection, which is unchanged (Wo stationary, attn^T moving).
 - k/v projections are emitted just-in-time inside each block's attention
   (the diagonal kj tiles are the last consumers), giving the PE fill work
   during the ACT-paced exp stretches of late blocks; q projections stay
   ahead of their block. Out-projection of block qb-1 is emitted after
   attention(qb) as lower-priority fill (v1 pattern).
 - yt evictions run on the otherwise-idle Pool engine; output DMAs go on
   the SP queue so the ACT sequencer never stalls on DMA config.
"""
import numpy as np

import concourse.bacc as bacc
import concourse.tile as tile
from concourse import mybir
from concourse.bass import broadcast_tensor_aps
from concourse.bass_utils import run_bass_kernel_spmd

B, L, D = 4, 2048, 1024
NH, HD = 16, 64
G = 8            # heads per core
NP = G // 2      # head pairs per core
LC = 512         # qi block
KT = 128         # kj tile
NKJ = L // KT    # 16
NLC = L // LC    # 4 qi blocks
CH = 256         # qkv l-chunk
NKT = D // 128   # 8 contraction tiles
F32 = mybir.dt.float32
BF = mybir.dt.bfloat16
AF = mybir.ActivationFunctionType

_cache = {}


def _build(trace_names=False):
    nc = bacc.Bacc("TRN2", target_bir_lowering=False, debug=False, num_devices=8)
    xT = nc.dram_tensor("xT", [NKT, 128, L], BF, kind="ExternalInput")
    W_in = nc.dram_tensor("W_in", [NKT, 128, 3 * G * HD], BF,
                          kind="ExternalInput")
    W_out_s = nc.dram_tensor("W_out_s", [NP, 128, D], BF, kind="ExternalInput")
    masks = nc.dram_tensor("masks", [128, 256], BF, kind="ExternalInput")
    yT = nc.dram_tensor("yT", [D, L], F32, kind="ExternalOutput")

    scale = float(1.0 / np.sqrt(HD))
    WG = 256                  # W dma col-group width
    NWG = (3 * G * HD) // WG  # 6 groups

    with tile.TileContext(nc) as tc:
        with tc.tile_pool(name="store", bufs=1) as store, \
             tc.tile_pool(name="qtp", bufs=2) as qtp, \
             tc.tile_pool(name="xtp", bufs=8) as xtp, \
             tc.tile_pool(name="expp", bufs=34) as expp, \
             tc.tile_pool(name="attnp", bufs=2) as attnp, \
             tc.tile_pool(name="atp", bufs=4) as atp, \
             tc.tile_pool(name="denp", bufs=2) as denp, \
             tc.tile_pool(name="ytp", bufs=3) as ytp, \
             tc.tile_pool(name="mm_ps", bufs=4, space="PSUM") as mm_ps, \
             tc.tile_pool(name="scores", bufs=2, space="PSUM") as scores_p:
            W_sb = store.tile([128, NKT, 3 * G * HD], BF)
            Wo_sb = store.tile([128, NP, D], BF)
            kT_sb = store.tile([128, NP, L], BF)
            v_sb = store.tile([128, NKJ, G, HD + 1], BF)
            mi_sb = store.tile([128, 256], BF)
            id_sb = mi_sb[:, 0:128]
            masks_sb = mi_sb[:, 128:256]

            nc.vector.memset(v_sb[:, :, :, HD:HD + 1], 1.0)

            xT_r = xT.rearrange("kt p l -> p kt l")
            W_r = W_in.rearrange("kt p c -> p kt c")
            yT_r = yT.rearrange("(m p) l -> p m l", p=128)

            # DMA order = first-use order: xt0, W[q m01], W[k m01],
            # ident, xt1, masks, W[q m23], W[k m23], W[v], Wo.  Startup
            # loads go on the scalar queue (ACT idle then); mid-kernel x
            # chunks and yt outputs go on the SP queue so the ACT
            # sequencer never stalls on DMA config mid-exp-stream.
            xt_pre = [xtp.tile([128, NKT, CH], BF, name=f"xt{c}", tag="xt")
                      for c in range(2)]

            def wdma(g):
                nc.scalar.dma_start(out=W_sb[:, :, g * WG:(g + 1) * WG],
                                    in_=W_r[:, :, g * WG:(g + 1) * WG])

            nc.sync.dma_start(out=xt_pre[0][:], in_=xT_r[:, :, 0:CH])
            wdma(0)
            nc.gpsimd.dma_start(out=mi_sb[:], in_=masks[:])
            wdma(2)
            nc.sync.dma_start(out=xt_pre[1][:],
                               in_=xT_r[:, :, CH:2 * CH])
            wdma(1)
            wdma(3)
            wdma(4)
            wdma(5)
            nc.scalar.dma_start(
                out=Wo_sb[:], in_=W_out_s.rearrange("ct p d -> p ct d"))

            # p-state warmup: keep the PE continuously busy from t~0 so
            # it reaches full clock before the first real matmul's inputs
            # arrive (the first ~5us are DMA-bound anyway)
            warm = store.tile([128, 128], BF)
            nc.vector.memset(warm[:], 0.25)
            wps = mm_ps.tile([128, 128], F32, tag="ps", name="wps")
            for _ in range(48):
                nc.tensor.matmul(wps[:], warm[:], warm[:], start=True,
                                 stop=True)

            xts = {}

            def get_xt(c):
                if c not in xts:
                    if c < 2:
                        xts[c] = xt_pre[c]
                    else:
                        xt = xtp.tile([128, NKT, CH], BF, name=f"xt{c}",
                                      tag="xt")
                        nc.sync.dma_start(out=xt[:],
                                          in_=xT_r[:, :, c * CH:(c + 1) * CH])
                        xts[c] = xt
                return xts[c]

            KOFF = G * HD
            VOFF = 2 * G * HD

            def qk_m(c, m, qT_blk):
                """q and k projections for one m (head-pair) tile of one
                x chunk — the minimal unit on the first block's critical
                path."""
                xt = get_xt(c)
                half = (c % 2) * CH
                for off, out_ap in (
                        (m * 128, qT_blk[:, m, half:half + CH]),
                        (KOFF + m * 128, kT_sb[:, m, c * CH:c * CH + CH])):
                    ps = mm_ps.tile([128, CH], F32, tag="ps", name="psqk")
                    for kt in range(NKT):
                        nc.tensor.matmul(
                            ps[:], W_sb[:, kt, off:off + 128],
                            xt[:, kt, :], start=(kt == 0), stop=(kt == NKT - 1))
                    nc.vector.tensor_copy(out=out_ap, in_=ps[:])

            def v_proj(c):
                xt = get_xt(c)
                for sub in range(CH // KT):
                    ps = mm_ps.tile([128, G * HD], F32, tag="ps", name="psv")
                    for kt in range(NKT):
                        nc.tensor.matmul(
                            ps[:], xt[:, kt, sub * KT:(sub + 1) * KT],
                            W_sb[:, kt, VOFF:VOFF + G * HD],
                            start=(kt == 0), stop=(kt == NKT - 1))
                    nc.vector.tensor_copy(
                        out=v_sb[:, c * (CH // KT) + sub, :, 0:HD],
                        in_=ps[:].rearrange("p (h d) -> p h d", h=G))

            def q_parts(c, m, qT_blk):
                """Two ~426ns queue items sharing one open PSUM group, so
                a ready score matmul never waits behind a long fill item."""
                st = {}
                half = (c % 2) * CH

                def p0():
                    xt = get_xt(c)
                    st["ps"] = mm_ps.tile([128, CH], F32, tag="ps", name="psq")
                    for kt in range(4):
                        nc.tensor.matmul(
                            st["ps"][:], W_sb[:, kt, m * 128:(m + 1) * 128],
                            xt[:, kt, :], start=(kt == 0), stop=False)

                def p1():
                    xt = get_xt(c)
                    for kt in range(4, NKT):
                        nc.tensor.matmul(
                            st["ps"][:], W_sb[:, kt, m * 128:(m + 1) * 128],
                            xt[:, kt, :], start=False, stop=(kt == NKT - 1))
                    nc.vector.tensor_copy(out=qT_blk[:, m, half:half + CH],
                                          in_=st["ps"][:])
                return [(CH, p0), (CH, p1)]

            def k_parts(c, m):
                st = {}

                def p0():
                    xt = get_xt(c)
                    st["ps"] = mm_ps.tile([128, CH], F32, tag="ps", name="psk")
                    for kt in range(4):
                        nc.tensor.matmul(
                            st["ps"][:],
                            W_sb[:, kt, KOFF + m * 128:KOFF + (m + 1) * 128],
                            xt[:, kt, :], start=(kt == 0), stop=False)

                def p1():
                    xt = get_xt(c)
                    for kt in range(4, NKT):
                        nc.tensor.matmul(
                            st["ps"][:],
                            W_sb[:, kt, KOFF + m * 128:KOFF + (m + 1) * 128],
                            xt[:, kt, :], start=False, stop=(kt == NKT - 1))
                    nc.vector.tensor_copy(
                        out=kT_sb[:, m, c * CH:c * CH + CH], in_=st["ps"][:])
                return [(CH, p0), (CH, p1)]

            def v_parts(c, sub):
                st = {}

                def mk(kt0, kt1, first, last):
                    def p():
                        xt = get_xt(c)
                        if first:
                            st["ps"] = mm_ps.tile([128, G * HD], F32,
                                                  tag="ps", name="psv")
                        for kt in range(kt0, kt1):
                            nc.tensor.matmul(
                                st["ps"][:], xt[:, kt, sub * KT:(sub + 1) * KT],
                                W_sb[:, kt, VOFF:VOFF + G * HD],
                                start=(kt == 0), stop=(kt == NKT - 1))
                        if last:
                            nc.vector.tensor_copy(
                                out=v_sb[:, c * (CH // KT) + sub, :, 0:HD],
                                in_=st["ps"][:].rearrange("p (h d) -> p h d",
                                                          h=G))
                    return p
                return [(G * HD, mk(0, 2, True, False)),
                        (G * HD, mk(2, 4, False, False)),
                        (G * HD, mk(4, 6, False, False)),
                        (G * HD, mk(6, 8, False, True))]

            attn_nats = {}

            def av_seg(qb, pair, j, exs, st, t0, t1):
                """AV accumulation segment [t0, t1) for one qtile of a
                pair; the last segment normalizes out of PSUM."""
                hA, hB = 2 * pair, 2 * pair + 1
                jt = qb * (LC // KT) + j
                if t0 == 0:
                    if j == 0:
                        attn_nats[pair] = attnp.tile([128, 4, 128], BF,
                                                     name="anat")
                    st["avA"] = mm_ps.tile([128, LC], F32, tag="ps",
                                           name="avA")
                    st["avB"] = mm_ps.tile([128, LC], F32, tag="ps",
                                           name="avB")
                avA, avB = st["avA"], st["avB"]
                for t in range(t0, min(t1, jt + 1)):
                    nc.tensor.matmul(
                        avA[:, 0:HD + 1], exs[t][:, j * KT:(j + 1) * KT],
                        v_sb[:, t, hA, :], start=(t == 0), stop=(t == jt))
                    nc.tensor.matmul(
                        avB[:, 0:HD + 1],
                        exs[t][:, LC + j * KT:LC + (j + 1) * KT],
                        v_sb[:, t, hB, :], start=(t == 0), stop=(t == jt))
                if t1 < jt + 1:
                    return
                attn_nat = attn_nats[pair]
                den = denp.tile([128, 2], F32, name="den")
                for h2, av in ((0, avA), (1, avB)):
                    nc.vector.reciprocal(out=den[:, h2:h2 + 1],
                                         in_=av[:, HD:HD + 1])
                    nc.vector.tensor_scalar(
                        out=attn_nat[:, j, h2 * HD:(h2 + 1) * HD],
                        in0=av[:, 0:HD], scalar1=den[:, h2:h2 + 1],
                        scalar2=None, op0=mybir.AluOpType.mult)

            def av_group(qb, pair, j, exs):
                st = {}
                av_seg(qb, pair, j, exs, st, 0, 99)

            def transpose_j(pair, j, attnT):
                attn_nat = attn_nats[pair]
                tp = mm_ps.tile([128, KT], BF, tag="ps", name="tp")
                nc.tensor.matmul(tp[:], attn_nat[:, j, :], id_sb,
                                 is_transpose=True)
                nc.vector.tensor_copy(
                    out=attnT[:, pair, j * KT:(j + 1) * KT], in_=tp[:])

            def transposes(pair, attnT):
                for j in range(4):
                    transpose_j(pair, j, attnT)
                attn_nats.pop(pair)

            def outproj_p0(qb, m, attnT, st):
                st["ps"] = mm_ps.tile([128, LC], F32, tag="ps", name="psy")
                for ct in range(2):
                    nc.tensor.matmul(
                        st["ps"][:], Wo_sb[:, ct, m * 128:(m + 1) * 128],
                        attnT[:, ct, :], start=(ct == 0), stop=False)

            def outproj_p1(qb, m, attnT, st):
                for ct in range(2, NP):
                    nc.tensor.matmul(
                        st["ps"][:], Wo_sb[:, ct, m * 128:(m + 1) * 128],
                        attnT[:, ct, :], start=False, stop=(ct == NP - 1))
                yt = ytp.tile([128, LC], F32, name="yt", tag="yt")
                nc.vector.tensor_copy(out=yt[:], in_=st["ps"][:])
                nc.sync.dma_start(out=yT_r[:, m, qb * LC:qb * LC + LC],
                                  in_=yt[:])

            yt_last = {}

            def outproj_mj(m, j, attnT, yts):
                ps = mm_ps.tile([128, KT], F32, tag="ps", name="psj")
                for ct in range(NP):
                    nc.tensor.matmul(
                        ps[:], Wo_sb[:, ct, m * 128:(m + 1) * 128],
                        attnT[:, ct, j * KT:(j + 1) * KT],
                        start=(ct == 0), stop=(ct == NP - 1))
                nc.vector.tensor_copy(out=yts[m][:, j * KT:(j + 1) * KT],
                                      in_=ps[:])

            def outproj_m(qb, m, attnT):
                ps = mm_ps.tile([128, LC], F32, tag="ps", name="psy")
                for ct in range(NP):
                    nc.tensor.matmul(
                        ps[:], Wo_sb[:, ct, m * 128:(m + 1) * 128],
                        attnT[:, ct, :], start=(ct == 0), stop=(ct == NP - 1))
                yt = ytp.tile([128, LC], F32, name="yt", tag="yt")
                nc.vector.tensor_copy(out=yt[:], in_=ps[:])
                nc.sync.dma_start(out=yT_r[:, m, qb * LC:qb * LC + LC],
                                  in_=yt[:])

            # Global fill queue: every PE task that is not on the ACT
            # critical path (scores+exp) is queued with a row-cost and
            # drained into the t-loops at the rate the exp stream frees PE
            # cycles (exp runs at 0.833 ns/col on ACT vs 0.4167 ns/row on
            # PE, so each exp column buys about one spare PE row beyond
            # the score matmuls).  Unspent inventory floats forward into
            # the ACT-heavy late blocks; dependency-forced items are
            # drained explicitly at pair/block boundaries.
            fillq = []
            opq = []   # out-projections: lowest priority, float late
            cur_qb = [0]

            def qfill(rows, fn, cls="", min_qb=-1):
                fns = fn if isinstance(fn, list) else [fn]
                fillq.append([rows, fns, cls, min_qb])

            def _pick(q, cls):
                for i, (rows, fns, c, mq) in enumerate(q):
                    if cls is not None:
                        if c == cls:
                            return i
                        continue
                    if mq <= cur_qb[0]:
                        return i
                return None

            def drain(budget=None, cls=None):
                while True:
                    if budget is not None and budget <= 0:
                        break
                    i = _pick(fillq, cls)
                    if i is not None:
                        rows, fns, c, mq = fillq.pop(i)
                    elif cls is None:
                        j = _pick(opq, None)
                        if j is None:
                            break
                        rows, fns, c, mq = opq.pop(j)
                    else:
                        break
                    for fn in fns:
                        fn()
                    if budget is not None:
                        budget -= rows

            def queue_proj(c, qT_blk):
                for m in range(NP):
                    parts = q_parts(c, m, qT_blk)
                    qfill(sum(r for r, _ in parts), [f for _, f in parts],
                          f"q{c}")
                for m in range(NP):
                    parts = k_parts(c, m)
                    qfill(sum(r for r, _ in parts), [f for _, f in parts],
                          f"kv{c}", c // 2)
                for sub in range(CH // KT):
                    parts = v_parts(c, sub)
                    qfill(sum(r for r, _ in parts), [f for _, f in parts],
                          f"kv{c}", c // 2)

            def queue_av(qb, pair, exs, attnT):
                for j in range(4):
                    jt = qb * (LC // KT) + j
                    qfill(2 * (HD + 1) * (jt + 1),
                          lambda j=j: av_group(qb, pair, j, exs),
                          f"av{pair}")
                for j in range(4):
                    qfill(KT, lambda j=j: transpose_j(pair, j, attnT),
                          f"av{pair}")
                qfill(0, lambda: attn_nats.pop(pair), f"av{pair}")

            def attention(qb, qT_blk, attnT, qT_next):
                n_t = (qb + 1) * (LC // KT)
                last = qb == NLC - 1
                if qT_next is not None:
                    queue_proj(2 * qb + 2, qT_next)
                    qfill(0, lambda: None)
                    queue_proj(2 * qb + 3, qT_next)
                if qb == 0:
                    # pair-0 critical pieces first, then the rest of
                    # chunk 0 (ready as soon as xt0 lands — fills the
                    # PE while xt1 is still in flight), then chunk 1
                    qk_m(0, 0, qT_blk)
                    qk_m(1, 0, qT_blk)
                    for m in range(1, NP):
                        qk_m(0, m, qT_blk)
                    for m in range(1, NP):
                        qk_m(1, m, qT_blk)
                for pair in range(NP):
                    exs = []
                    pace_own_av = last and pair == NP - 1
                    for t in range(n_t):
                        diag = t >= qb * (LC // KT)
                        o = t - qb * (LC // KT) if diag else 0
                        z = o * KT
                        if diag and o == 0 and qb > 0:
                            drain(cls=f"kv{2 * qb}")
                        if diag and o == 2 and qb > 0:
                            drain(cls=f"kv{2 * qb + 1}")
                        sc = scores_p.tile([128, 2 * LC], F32, tag="sc")
                        nc.tensor.matmul(
                            sc[:, z:LC],
                            kT_sb[0:64, pair, t * KT:(t + 1) * KT],
                            qT_blk[0:64, pair, z:LC], start=True,
                            stop=not diag)
                        nc.tensor.matmul(
                            sc[:, LC + z:2 * LC],
                            kT_sb[64:128, pair, t * KT:(t + 1) * KT],
                            qT_blk[64:128, pair, z:LC], start=True,
                            stop=not diag)
                        if diag:
                            nc.tensor.matmul(sc[:, z:z + KT], id_sb,
                                             masks_sb,
                                             start=False, stop=True)
                            nc.tensor.matmul(sc[:, LC + z:LC + z + KT],
                                             id_sb, masks_sb,
                                             start=False, stop=True)
                        ex = expp.tile([128, 2 * LC], BF)
                        sc_v = sc[:].rearrange("p (h c) -> p h c", h=2)[:, :, z:LC]
                        ex_v = ex[:].rearrange("p (h c) -> p h c", h=2)[:, :, z:LC]
                        nc.scalar.activation(ex_v, sc_v, AF.Exp, scale=scale)
                        exs.append(ex)
                        if pace_own_av and diag:
                            av_group(qb, pair, o, exs)
                        spare = 2 * (LC - z) - (256 if diag else 0)
                        drain(budget=int(spare * 0.75))
                    # dependency-forced drains: the pair-before-last's AV
                    # must complete (exp-tile pool bound) ...
                    if pair >= 1:
                        drain(cls=f"av{pair - 1}")
                    if pace_own_av:
                        transposes(pair, attnT)
                    else:
                        queue_av(qb, pair, exs, attnT)
                # ... and the next block's q projections before its
                # scores (k/v only feed its diagonal tiles: drained there)
                if qT_next is not None:
                    drain(cls=f"q{2 * qb + 2}")
                    drain(cls=f"q{2 * qb + 3}")
                if not last:
                    drain(cls=f"av{NP - 1}")

            attnTs = {}
            qTs = {0: qtp.tile([128, NP, LC], BF, name="qT0", tag="qT")}
            for qb in range(NLC):
                cur_qb[0] = qb
                if qb + 1 < NLC:
                    qTs[qb + 1] = qtp.tile([128, NP, LC], BF,
                                           name=f"qT{qb + 1}", tag="qT")
                attnTs[qb] = atp.tile([128, NP, LC], BF, name=f"aT{qb}",
                                      tag="aT")
                if qb == 0:
                    for c in range(2):
                        for sub in range(2):
                            parts = v_parts(c, sub)
                            qfill(sum(r for r, _ in parts),
                                  [f for _, f in parts], f"kv{c}")
                attention(qb, qTs[qb], attnTs[qb], qTs.get(qb + 1))
                if qb < NLC - 1:
                    for m in range(D // 128):
                        opq.append([NP * LC,
                                    [lambda m=m, qb=qb:
                                     outproj_m(qb, m, attnTs[qb])], "op",
                                    NLC - 1])
            cur_qb[0] = NLC
            drain()
            for m in range(D // 128):
                outproj_m(NLC - 1, m, attnTs[NLC - 1])
    nc.compile()
    return nc


def _make_masks():
    import ml_dtypes
    r = np.arange(128)[:, None]
    c = np.arange(128)[None, :]
    return np.where(c >= r, 0.0, -1e6).astype(ml_dtypes.bfloat16)


def _make_ident():
    import ml_dtypes
    return np.eye(128, dtype=ml_dtypes.bfloat16)


def kernel(x, W_qkv, b_qkv, W_out, b_out, _trace=False, _trace_kwargs=None):
    import ml_dtypes
    bf16 = ml_dtypes.bfloat16
    x = np.ascontiguousarray(x, dtype=np.float32)
    W_qkv = np.asarray(W_qkv, dtype=np.float32)
    b_qkv = np.asarray(b_qkv, dtype=np.float32)
    W_out = np.asarray(W_out, dtype=np.float32)
    b_out = np.asarray(b_out, dtype=np.float32)
    assert np.all(b_qkv == 0.0), "nonzero b_qkv not supported by this kernel"

    if "nc" not in _cache:
        _cache["nc"] = _build()
    nc = _cache["nc"]

    masks = _make_masks()
    ident = _make_ident()
    Wq, Wk, Wv = W_qkv[:, 0:D], W_qkv[:, D:2 * D], W_qkv[:, 2 * D:3 * D]

    in_maps = []
    for c in range(8):
        b, g = divmod(c, 2)
        cols = slice(g * G * HD, (g + 1) * G * HD)
        W_in = np.concatenate([Wq[:, cols], Wk[:, cols], Wv[:, cols]], axis=1)
        in_maps.append({
            "xT": np.ascontiguousarray(x[b].T).astype(bf16).reshape(
                NKT, 128, L),
            "W_in": W_in.astype(bf16).reshape(NKT, 128, 3 * G * HD),
            "W_out_s": W_out[cols, :].astype(bf16).reshape(NP, 128, D),
            "masks": np.concatenate([ident, masks], axis=1),
        })

    kw = {}
    if _trace:
        kw["trace"] = True
        kw.update(_trace_kwargs or {})
    res = run_bass_kernel_spmd(nc, in_maps, list(range(8)), **kw)

    out = np.empty((B, L, D), dtype=np.float32)
    for b in range(B):
        yT = res.results[2 * b]["yT"] + res.results[2 * b + 1]["yT"]
        out[b] = yT.T + b_out
    if _trace:
        _cache["last_result"] = res
    return out


# revision 63
# speedup vs baseline: 1.0110x; 1.0103x over previous
"""Causal multi-head self-attention on 8 Trainium2 NeuronCores (Bass/Tile).

Problem (hardcoded): x [4, 2048, 1024] fp32, W_qkv [1024, 3072], b_qkv [3072],
W_out [1024, 1024], b_out [1024]. 16 heads, head_dim 64.

Sharding: core c = 2*b + g handles batch b (4 batches) and head group g
(8 heads): tensor-parallel over heads within a batch pair. Each core computes
qkv projection for its 8 heads, causal flash attention, and a partial output
projection (its 512 rows of W_out). The two partials per batch are summed on
the host (the "all-reduce") along with b_out.

Design notes (v2):
 - all matmul operands are bf16 (1 cyc/row on the PE at any moving width;
   fp32r would drop to 4 cyc/row for the narrow diagonal tiles). PSUM
   accumulation stays fp32, so only input quantization (~0.4%) is lost.
 - scores^T tiles [kj, qi] as in v1 (z-skip of fully-masked 128-col groups,
   -1e6 causal bias added via identity x mask matmul on the diagonal tiles,
   exp without max-subtraction on ACT, output bf16).
 - AV uses the ex tile as the *stationary* and v [128kj, 65] as the moving:
   cost is 65 rows per (head, qtile, kj-tile) instead of 128 — about half
   the moving rows of the v1 orientation. Output lands naturally as
   [q, hd] with the appended ones-column giving the softmax denominator in
   column 64, so normalization is a per-partition reciprocal + broadcast
   multiply on DVE (no PE broadcast matmuls at all).
 - normalized attention tiles are transposed back to [hd, q] through the PE
   (pure transpose against a bf16 identity, 128 rows each) to feed the
   output projection, which is unchanged (Wo stationary, attn^T moving).
 - k/v projections are emitted just-in-time inside each block's attention
   (the diagonal kj tiles are the last consumers), giving the PE fill work
   during the ACT-paced exp stretches of late blocks; q projections stay
   ahead of their block. Out-projection of block qb-1 is emitted after
   attention(qb) as lower-priority fill (v1 pattern).
 - yt evictions run on the otherwise-idle Pool engine; output DMAs go on
   the SP queue so the ACT sequencer never stalls on DMA config.
"""
import numpy as np

import concourse.bacc as bacc
import concourse.tile as tile
from concourse import mybir
from concourse.bass import broadcast_tensor_aps
from concourse.bass_utils import run_bass_kernel_spmd

B, L, D = 4, 2048, 1024
NH, HD = 16, 64
G = 8            # heads per core
NP = G // 2      # head pairs per core
LC = 512         # qi block
KT = 128         # kj tile
NKJ = L // KT    # 16
NLC = L // LC    # 4 qi blocks
CH = 256         # qkv l-chunk
NKT = D // 128   # 8 contraction tiles
F32 = mybir.dt.float32
BF = mybir.dt.bfloat16
AF = mybir.ActivationFunctionType

_cache = {}


def _build(trace_names=False):
    nc = bacc.Bacc("TRN2", target_bir_lowering=False, debug=False, num_devices=8)
    xT = nc.dram_tensor("xT", [NKT, 128, L], BF, kind="ExternalInput")
    W_in = nc.dram_tensor("W_in", [NKT, 128, 3 * G * HD], BF,
                          kind="ExternalInput")
    W_out_s = nc.dram_tensor("W_out_s", [NP, 128, D], BF, kind="ExternalInput")
    masks = nc.dram_tensor("masks", [128, 256], BF, kind="ExternalInput")
    yT = nc.dram_tensor("yT", [D, L], F32, kind="ExternalOutput")

    scale = float(1.0 / np.sqrt(HD))
    WG = 256                  # W dma col-group width
    NWG = (3 * G * HD) // WG  # 6 groups

    with tile.TileContext(nc) as tc:
        with tc.tile_pool(name="store", bufs=1) as store, \
             tc.tile_pool(name="qtp", bufs=2) as qtp, \
             tc.tile_pool(name="xtp", bufs=8) as xtp, \
             tc.tile_pool(name="expp", bufs=34) as expp, \
             tc.tile_pool(name="attnp", bufs=2) as attnp, \
             tc.tile_pool(name="atp", bufs=4) as atp, \
             tc.tile_pool(name="denp", bufs=2) as denp, \
             tc.tile_pool(name="ytp", bufs=5) as ytp, \
             tc.tile_pool(name="mm_ps", bufs=4, space="PSUM") as mm_ps, \
             tc.tile_pool(name="scores", bufs=2, space="PSUM") as scores_p:
            W_sb = store.tile([128, NKT, 3 * G * HD], BF)
            Wo_sb = store.tile([128, NP, D], BF)
            kT_sb = store.tile([128, NP, L], BF)
            v_sb = store.tile([128, NKJ, G, HD + 1], BF)
            mi_sb = store.tile([128, 256], BF)
            id_sb = mi_sb[:, 0:128]
            masks_sb = mi_sb[:, 128:256]

            nc.vector.memset(v_sb[:, :, :, HD:HD + 1], 1.0)

            xT_r = xT.rearrange("kt p l -> p kt l")
            W_r = W_in.rearrange("kt p c -> p kt c")
            yT_r = yT.rearrange("(m p) l -> p m l", p=128)

            # DMA order = first-use order: xt0, W[q m01], W[k m01],
            # ident, xt1, masks, W[q m23], W[k m23], W[v], Wo.  Startup
            # loads go on the scalar queue (ACT idle then); mid-kernel x
            # chunks and yt outputs go on the SP queue so the ACT
            # sequencer never stalls on DMA config mid-exp-stream.
            xt_pre = [xtp.tile([128, NKT, CH], BF, name=f"xt{c}", tag="xt")
                      for c in range(2)]

            def wdma(g):
                nc.scalar.dma_start(out=W_sb[:, :, g * WG:(g + 1) * WG],
                                    in_=W_r[:, :, g * WG:(g + 1) * WG])

            nc.sync.dma_start(out=xt_pre[0][:], in_=xT_r[:, :, 0:CH])
            wdma(0)
            nc.gpsimd.dma_start(out=mi_sb[:], in_=masks[:])
            wdma(2)
            nc.sync.dma_start(out=xt_pre[1][:],
                               in_=xT_r[:, :, CH:2 * CH])
            wdma(1)
            wdma(3)
            wdma(4)
            wdma(5)
            nc.scalar.dma_start(
                out=Wo_sb[:], in_=W_out_s.rearrange("ct p d -> p ct d"))

            # p-state warmup: keep the PE continuously busy from t~0 so
            # it reaches full clock before the first real matmul's inputs
            # arrive (the first ~5us are DMA-bound anyway)
            warm = store.tile([128, 128], BF)
            nc.vector.memset(warm[:], 0.25)
            wps = mm_ps.tile([128, 128], F32, tag="ps", name="wps")
            for _ in range(44):
                nc.tensor.matmul(wps[:], warm[:], warm[:], start=True,
                                 stop=True)

            xts = {}

            def get_xt(c):
                if c not in xts:
                    if c < 2:
                        xts[c] = xt_pre[c]
                    else:
                        xt = xtp.tile([128, NKT, CH], BF, name=f"xt{c}",
                                      tag="xt")
                        nc.sync.dma_start(out=xt[:],
                                          in_=xT_r[:, :, c * CH:(c + 1) * CH])
                        xts[c] = xt
                return xts[c]

            KOFF = G * HD
            VOFF = 2 * G * HD

            def qk_m(c, m, qT_blk):
                """q and k projections for one m (head-pair) tile of one
                x chunk — the minimal unit on the first block's critical
                path."""
                xt = get_xt(c)
                half = (c % 2) * CH
                for off, out_ap in (
                        (m * 128, qT_blk[:, m, half:half + CH]),
                        (KOFF + m * 128, kT_sb[:, m, c * CH:c * CH + CH])):
                    ps = mm_ps.tile([128, CH], F32, tag="ps", name="psqk")
                    for kt in range(NKT):
                        nc.tensor.matmul(
                            ps[:], W_sb[:, kt, off:off + 128],
                            xt[:, kt, :], start=(kt == 0), stop=(kt == NKT - 1))
                    nc.vector.tensor_copy(out=out_ap, in_=ps[:])

            def v_proj(c):
                xt = get_xt(c)
                for sub in range(CH // KT):
                    ps = mm_ps.tile([128, G * HD], F32, tag="ps", name="psv")
                    for kt in range(NKT):
                        nc.tensor.matmul(
                            ps[:], xt[:, kt, sub * KT:(sub + 1) * KT],
                            W_sb[:, kt, VOFF:VOFF + G * HD],
                            start=(kt == 0), stop=(kt == NKT - 1))
                    nc.vector.tensor_copy(
                        out=v_sb[:, c * (CH // KT) + sub, :, 0:HD],
                        in_=ps[:].rearrange("p (h d) -> p h d", h=G))

            def q_parts(c, m, qT_blk):
                """Two ~426ns queue items sharing one open PSUM group, so
                a ready score matmul never waits behind a long fill item."""
                st = {}
                half = (c % 2) * CH

                def p0():
                    xt = get_xt(c)
                    st["ps"] = mm_ps.tile([128, CH], F32, tag="ps", name="psq")
                    for kt in range(4):
                        nc.tensor.matmul(
                            st["ps"][:], W_sb[:, kt, m * 128:(m + 1) * 128],
                            xt[:, kt, :], start=(kt == 0), stop=False)

                def p1():
                    xt = get_xt(c)
                    for kt in range(4, NKT):
                        nc.tensor.matmul(
                            st["ps"][:], W_sb[:, kt, m * 128:(m + 1) * 128],
                            xt[:, kt, :], start=False, stop=(kt == NKT - 1))
                    nc.vector.tensor_copy(out=qT_blk[:, m, half:half + CH],
                                          in_=st["ps"][:])
                return [(CH, p0), (CH, p1)]

            def k_parts(c, m):
                st = {}

                def p0():
                    xt = get_xt(c)
                    st["ps"] = mm_ps.tile([128, CH], F32, tag="ps", name="psk")
                    for kt in range(4):
                        nc.tensor.matmul(
                            st["ps"][:],
                            W_sb[:, kt, KOFF + m * 128:KOFF + (m + 1) * 128],
                            xt[:, kt, :], start=(kt == 0), stop=False)

                def p1():
                    xt = get_xt(c)
                    for kt in range(4, NKT):
                        nc.tensor.matmul(
                            st["ps"][:],
                            W_sb[:, kt, KOFF + m * 128:KOFF + (m + 1) * 128],
                            xt[:, kt, :], start=False, stop=(kt == NKT - 1))
                    nc.vector.tensor_copy(
                        out=kT_sb[:, m, c * CH:c * CH + CH], in_=st["ps"][:])
                return [(CH, p0), (CH, p1)]

            def v_parts(c, sub):
                st = {}

                def mk(kt0, kt1, first, last):
                    def p():
                        xt = get_xt(c)
                        if first:
                            st["ps"] = mm_ps.tile([128, G * HD], F32,
                                                  tag="ps", name="psv")
                        for kt in range(kt0, kt1):
                            nc.tensor.matmul(
                                st["ps"][:], xt[:, kt, sub * KT:(sub + 1) * KT],
                                W_sb[:, kt, VOFF:VOFF + G * HD],
                                start=(kt == 0), stop=(kt == NKT - 1))
                        if last:
                            nc.vector.tensor_copy(
                                out=v_sb[:, c * (CH // KT) + sub, :, 0:HD],
                                in_=st["ps"][:].rearrange("p (h d) -> p h d",
                                                          h=G))
                    return p
                return [(G * HD, mk(0, 2, True, False)),
                        (G * HD, mk(2, 4, False, False)),
                        (G * HD, mk(4, 6, False, False)),
                        (G * HD, mk(6, 8, False, True))]

            attn_nats = {}

            def av_seg(qb, pair, j, exs, st, t0, t1):
                """AV accumulation segment [t0, t1) for one qtile of a
                pair; the last segment normalizes out of PSUM."""
                hA, hB = 2 * pair, 2 * pair + 1
                jt = qb * (LC // KT) + j
                if t0 == 0:
                    if j == 0:
                        attn_nats[pair] = attnp.tile([128, 4, 128], BF,
                                                     name="anat")
                    st["avA"] = mm_ps.tile([128, LC], F32, tag="ps",
                                           name="avA")
                    st["avB"] = mm_ps.tile([128, LC], F32, tag="ps",
                                           name="avB")
                avA, avB = st["avA"], st["avB"]
                for t in range(t0, min(t1, jt + 1)):
                    nc.tensor.matmul(
                        avA[:, 0:HD + 1], exs[t][:, j * KT:(j + 1) * KT],
                        v_sb[:, t, hA, :], start=(t == 0), stop=(t == jt))
                    nc.tensor.matmul(
                        avB[:, 0:HD + 1],
                        exs[t][:, LC + j * KT:LC + (j + 1) * KT],
                        v_sb[:, t, hB, :], start=(t == 0), stop=(t == jt))
                if t1 < jt + 1:
                    return
                attn_nat = attn_nats[pair]
                den = denp.tile([128, 2], F32, name="den")
                for h2, av in ((0, avA), (1, avB)):
                    nc.vector.reciprocal(out=den[:, h2:h2 + 1],
                                         in_=av[:, HD:HD + 1])
                    nc.vector.tensor_scalar(
                        out=attn_nat[:, j, h2 * HD:(h2 + 1) * HD],
                        in0=av[:, 0:HD], scalar1=den[:, h2:h2 + 1],
                        scalar2=None, op0=mybir.AluOpType.mult)

            def av_group(qb, pair, j, exs):
                st = {}
                av_seg(qb, pair, j, exs, st, 0, 99)

            def transpose_j(pair, j, attnT):
                attn_nat = attn_nats[pair]
                tp = mm_ps.tile([128, KT], BF, tag="ps", name="tp")
                nc.tensor.matmul(tp[:], attn_nat[:, j, :], id_sb,
                                 is_transpose=True)
                nc.vector.tensor_copy(
                    out=attnT[:, pair, j * KT:(j + 1) * KT], in_=tp[:])

            def transposes(pair, attnT):
                for j in range(4):
                    transpose_j(pair, j, attnT)
                attn_nats.pop(pair)

            def outproj_p0(qb, m, attnT, st):
                st["ps"] = mm_ps.tile([128, LC], F32, tag="ps", name="psy")
                for ct in range(2):
                    nc.tensor.matmul(
                        st["ps"][:], Wo_sb[:, ct, m * 128:(m + 1) * 128],
                        attnT[:, ct, :], start=(ct == 0), stop=False)

            def outproj_p1(qb, m, attnT, st):
                for ct in range(2, NP):
                    nc.tensor.matmul(
                        st["ps"][:], Wo_sb[:, ct, m * 128:(m + 1) * 128],
                        attnT[:, ct, :], start=False, stop=(ct == NP - 1))
                yt = ytp.tile([128, LC], F32, name="yt", tag="yt")
                nc.vector.tensor_copy(out=yt[:], in_=st["ps"][:])
                nc.sync.dma_start(out=yT_r[:, m, qb * LC:qb * LC + LC],
                                  in_=yt[:])

            yt_last = {}

            def outproj_mj(m, j, attnT, yts):
                ps = mm_ps.tile([128, KT], F32, tag="ps", name="psj")
                for ct in range(NP):
                    nc.tensor.matmul(
                        ps[:], Wo_sb[:, ct, m * 128:(m + 1) * 128],
                        attnT[:, ct, j * KT:(j + 1) * KT],
                        start=(ct == 0), stop=(ct == NP - 1))
                nc.vector.tensor_copy(out=yts[m][:, j * KT:(j + 1) * KT],
                                      in_=ps[:])

            def outproj_m(qb, m, attnT):
                ps = mm_ps.tile([128, LC], F32, tag="ps", name="psy")
                for ct in range(NP):
                    nc.tensor.matmul(
                        ps[:], Wo_sb[:, ct, m * 128:(m + 1) * 128],
                        attnT[:, ct, :], start=(ct == 0), stop=(ct == NP - 1))
                yt = ytp.tile([128, LC], F32, name="yt", tag="yt")
                nc.vector.tensor_copy(out=yt[:], in_=ps[:])
                nc.sync.dma_start(out=yT_r[:, m, qb * LC:qb * LC + LC],
                                  in_=yt[:])

            # Global fill queue: every PE task that is not on the ACT
            # critical path (scores+exp) is queued with a row-cost and
            # drained into the t-loops at the rate the exp stream frees PE
            # cycles (exp runs at 0.833 ns/col on ACT vs 0.4167 ns/row on
            # PE, so each exp column buys about one spare PE row beyond
            # the score matmuls).  Unspent inventory floats forward into
            # the ACT-heavy late blocks; dependency-forced items are
            # drained explicitly at pair/block boundaries.
            fillq = []
            opq = []   # out-projections: lowest priority, float late
            cur_qb = [0]

            def qfill(rows, fn, cls="", min_qb=-1):
                fns = fn if isinstance(fn, list) else [fn]
                fillq.append([rows, fns, cls, min_qb])

            def _pick(q, cls):
                for i, (rows, fns, c, mq) in enumerate(q):
                    if cls is not None:
                        if c == cls:
                            return i
                        continue
                    if mq <= cur_qb[0]:
                        return i
                return None

            def drain(budget=None, cls=None):
                while True:
                    if budget is not None and budget <= 0:
                        break
                    i = _pick(fillq, cls)
                    if i is not None:
                        rows, fns, c, mq = fillq.pop(i)
                    elif cls is None:
                        j = _pick(opq, None)
                        if j is None:
                            break
                        rows, fns, c, mq = opq.pop(j)
                    else:
                        break
                    for fn in fns:
                        fn()
                    if budget is not None:
                        budget -= rows

            def queue_proj(c, qT_blk):
                for m in range(NP):
                    parts = q_parts(c, m, qT_blk)
                    qfill(sum(r for r, _ in parts), [f for _, f in parts],
                          f"q{c}")
                for m in range(NP):
                    parts = k_parts(c, m)
                    qfill(sum(r for r, _ in parts), [f for _, f in parts],
                          f"kv{c}", c // 2)
                for sub in range(CH // KT):
                    parts = v_parts(c, sub)
                    qfill(sum(r for r, _ in parts), [f for _, f in parts],
                          f"kv{c}", c // 2)

            def queue_av(qb, pair, exs, attnT):
                for j in range(4):
                    jt = qb * (LC // KT) + j
                    qfill(2 * (HD + 1) * (jt + 1),
                          lambda j=j: av_group(qb, pair, j, exs),
                          f"av{pair}")
                for j in range(4):
                    qfill(KT, lambda j=j: transpose_j(pair, j, attnT),
                          f"av{pair}")
                qfill(0, lambda: attn_nats.pop(pair), f"av{pair}")

            def attention(qb, qT_blk, attnT, qT_next):
                n_t = (qb + 1) * (LC // KT)
                last = qb == NLC - 1
                if qT_next is not None:
                    queue_proj(2 * qb + 2, qT_next)
                    qfill(0, lambda: None)
                    queue_proj(2 * qb + 3, qT_next)
                if qb == 0:
                    # pair-0 critical pieces first, then the rest of
                    # chunk 0 (ready as soon as xt0 lands — fills the
                    # PE while xt1 is still in flight), then chunk 1
                    qk_m(0, 0, qT_blk)
                    qk_m(1, 0, qT_blk)
                    for m in range(1, NP):
                        qk_m(0, m, qT_blk)
                    for m in range(1, NP):
                        qk_m(1, m, qT_blk)
                for pair in range(NP):
                    exs = []
                    pace_own_av = last and pair == NP - 1
                    for t in range(n_t):
                        diag = t >= qb * (LC // KT)
                        o = t - qb * (LC // KT) if diag else 0
                        z = o * KT
                        if diag and o == 0 and qb > 0:
                            drain(cls=f"kv{2 * qb}")
                        if diag and o == 2 and qb > 0:
                            drain(cls=f"kv{2 * qb + 1}")
                        sc = scores_p.tile([128, 2 * LC], F32, tag="sc")
                        nc.tensor.matmul(
                            sc[:, z:LC],
                            kT_sb[0:64, pair, t * KT:(t + 1) * KT],
                            qT_blk[0:64, pair, z:LC], start=True,
                            stop=not diag)
                        nc.tensor.matmul(
                            sc[:, LC + z:2 * LC],
                            kT_sb[64:128, pair, t * KT:(t + 1) * KT],
                            qT_blk[64:128, pair, z:LC], start=True,
                            stop=not diag)
                        if diag:
                            nc.tensor.matmul(sc[:, z:z + KT], id_sb,
                                             masks_sb,
                                             start=False, stop=True)
                            nc.tensor.matmul(sc[:, LC + z:LC + z + KT],
                                             id_sb, masks_sb,
                                             start=False, stop=True)
                        ex = expp.tile([128, 2 * LC], BF)
                        sc_v = sc[:].rearrange("p (h c) -> p h c", h=2)[:, :, z:LC]
                        ex_v = ex[:].rearrange("p (h c) -> p h c", h=2)[:, :, z:LC]
                        nc.scalar.activation(ex_v, sc_v, AF.Exp, scale=scale)
                        exs.append(ex)
                        if pace_own_av and diag:
                            av_group(qb, pair, o, exs)
                        spare = 2 * (LC - z) - (256 if diag else 0)
                        drain(budget=int(spare * 0.75))
                    # dependency-forced drains: the pair-before-last's AV
                    # must complete (exp-tile pool bound) ...
                    if pair >= 1:
                        drain(cls=f"av{pair - 1}")
                    if pace_own_av:
                        transposes(pair, attnT)
                    else:
                        queue_av(qb, pair, exs, attnT)
                # ... and the next block's q projections before its
                # scores (k/v only feed its diagonal tiles: drained there)
                if qT_next is not None:
                    drain(cls=f"q{2 * qb + 2}")
                    drain(cls=f"q{2 * qb + 3}")
                if not last:
                    drain(cls=f"av{NP - 1}")

            attnTs = {}
            qTs = {0: qtp.tile([128, NP, LC], BF, name="qT0", tag="qT")}
            for qb in range(NLC):
                cur_qb[0] = qb
                if qb + 1 < NLC:
                    qTs[qb + 1] = qtp.tile([128, NP, LC], BF,
                                           name=f"qT{qb + 1}", tag="qT")
                attnTs[qb] = atp.tile([128, NP, LC], BF, name=f"aT{qb}",
                                      tag="aT")
                if qb == 0:
                    for c in range(2):
                        for sub in range(2):
                            parts = v_parts(c, sub)
                            qfill(sum(r for r, _ in parts),
                                  [f for _, f in parts], f"kv{c}")
                attention(qb, qTs[qb], attnTs[qb], qTs.get(qb + 1))
                if qb < NLC - 1:
                    for m in range(D // 128):
                        opq.append([NP * LC,
                                    [lambda m=m, qb=qb:
                                     outproj_m(qb, m, attnTs[qb])], "op",
                                    NLC - 1])
            cur_qb[0] = NLC
            drain()
            for m in range(D // 128):
                outproj_m(NLC - 1, m, attnTs[NLC - 1])
    nc.compile()
    return nc


def _make_masks():
    import ml_dtypes
    r = np.arange(128)[:, None]
    c = np.arange(128)[None, :]
    return np.where(c >= r, 0.0, -1e6).astype(ml_dtypes.bfloat16)


def _make_ident():
    import ml_dtypes
    return np.eye(128, dtype=ml_dtypes.bfloat16)


def kernel(x, W_qkv, b_qkv, W_out, b_out, _trace=False, _trace_kwargs=None):
    import ml_dtypes
    bf16 = ml_dtypes.bfloat16
    x = np.ascontiguousarray(x, dtype=np.float32)
    W_qkv = np.asarray(W_qkv, dtype=np.float32)
    b_qkv = np.asarray(b_qkv, dtype=np.float32)
    W_out = np.asarray(W_out, dtype=np.float32)
    b_out = np.asarray(b_out, dtype=np.float32)
    assert np.all(b_qkv == 0.0), "nonzero b_qkv not supported by this kernel"

    if "nc" not in _cache:
        _cache["nc"] = _build()
    nc = _cache["nc"]

    masks = _make_masks()
    ident = _make_ident()
    Wq, Wk, Wv = W_qkv[:, 0:D], W_qkv[:, D:2 * D], W_qkv[:, 2 * D:3 * D]

    in_maps = []
    for c in range(8):
        b, g = divmod(c, 2)
        cols = slice(g * G * HD, (g + 1) * G * HD)
        W_in = np.concatenate([Wq[:, cols], Wk[:, cols], Wv[:, cols]], axis=1)
        in_maps.append({
            "xT": np.ascontiguousarray(x[b].T).astype(bf16).reshape(
                NKT, 128, L),
            "W_in": W_in.astype(bf16).reshape(NKT, 128, 3 * G * HD),
            "W_out_s": W_out[cols, :].astype(bf16).reshape(NP, 128, D),
            "masks": np.concatenate([ident, masks], axis=1),
        })

    kw = {}
    if _trace:
        kw["trace"] = True
        kw.update(_trace_kwargs or {})
    res = run_bass_kernel_spmd(nc, in_maps, list(range(8)), **kw)

    out = np.empty((B, L, D), dtype=np.float32)
    for b in range(B):
        yT = res.results[2 * b]["yT"] + res.results[2 * b + 1]["yT"]
        out[b] = yT.T + b_out
    if _trace:
        _cache["last_result"] = res
    return out


# revision 64
# speedup vs baseline: 1.0313x; 1.0201x over previous
"""Causal multi-head self-attention on 8 Trainium2 NeuronCores (Bass/Tile).

Problem (hardcoded): x [4, 2048, 1024] fp32, W_qkv [1024, 3072], b_qkv [3072],
W_out [1024, 1024], b_out [1024]. 16 heads, head_dim 64.

Sharding: core c = 2*b + g handles batch b (4 batches) and head group g
(8 heads): tensor-parallel over heads within a batch pair. Each core computes
qkv projection for its 8 heads, causal flash attention, and a partial output
projection (its 512 rows of W_out). The two partials per batch are summed on
the host (the "all-reduce") along with b_out.

Design notes (v2):
 - all matmul operands are bf16 (1 cyc/row on the PE at any moving width;
   fp32r would drop to 4 cyc/row for the narrow diagonal tiles). PSUM
   accumulation stays fp32, so only input quantization (~0.4%) is lost.
 - scores^T tiles [kj, qi] as in v1 (z-skip of fully-masked 128-col groups,
   -1e6 causal bias added via identity x mask matmul on the diagonal tiles,
   exp without max-subtraction on ACT, output bf16).
 - AV uses the ex tile as the *stationary* and v [128kj, 65] as the moving:
   cost is 65 rows per (head, qtile, kj-tile) instead of 128 — about half
   the moving rows of the v1 orientation. Output lands naturally as
   [q, hd] with the appended ones-column giving the softmax denominator in
   column 64, so normalization is a per-partition reciprocal + broadcast
   multiply on DVE (no PE broadcast matmuls at all).
 - normalized attention tiles are transposed back to [hd, q] through the PE
   (pure transpose against a bf16 identity, 128 rows each) to feed the
   output projection, which is unchanged (Wo stationary, attn^T moving).
 - k/v projections are emitted just-in-time inside each block's attention
   (the diagonal kj tiles are the last consumers), giving the PE fill work
   during the ACT-paced exp stretches of late blocks; q projections stay
   ahead of their block. Out-projection of block qb-1 is emitted after
   attention(qb) as lower-priority fill (v1 pattern).
 - yt evictions run on the otherwise-idle Pool engine; output DMAs go on
   the SP queue so the ACT sequencer never stalls on DMA config.
"""
import numpy as np

import concourse.bacc as bacc
import concourse.tile as tile
from concourse import mybir
from concourse.bass import broadcast_tensor_aps
from concourse.bass_utils import run_bass_kernel_spmd

B, L, D = 4, 2048, 1024
NH, HD = 16, 64
G = 8            # heads per core
NP = G // 2      # head pairs per core
LC = 512         # qi block
KT = 128         # kj tile
NKJ = L // KT    # 16
NLC = L // LC    # 4 qi blocks
CH = 256         # qkv l-chunk
NKT = D // 128   # 8 contraction tiles
F32 = mybir.dt.float32
BF = mybir.dt.bfloat16
AF = mybir.ActivationFunctionType

_cache = {}


def _build(trace_names=False):
    nc = bacc.Bacc("TRN2", target_bir_lowering=False, debug=False, num_devices=8)
    xT = nc.dram_tensor("xT", [NKT, 128, L], BF, kind="ExternalInput")
    W_in = nc.dram_tensor("W_in", [NKT, 128, 3 * G * HD], BF,
                          kind="ExternalInput")
    W_out_s = nc.dram_tensor("W_out_s", [NP, 128, D], BF, kind="ExternalInput")
    masks = nc.dram_tensor("masks", [128, 256], BF, kind="ExternalInput")
    yT = nc.dram_tensor("yT", [D, L], F32, kind="ExternalOutput")

    scale = float(1.0 / np.sqrt(HD))
    WG = 256                  # W dma col-group width
    NWG = (3 * G * HD) // WG  # 6 groups

    with tile.TileContext(nc) as tc:
        with tc.tile_pool(name="store", bufs=1) as store, \
             tc.tile_pool(name="qtp", bufs=2) as qtp, \
             tc.tile_pool(name="xtp", bufs=8) as xtp, \
             tc.tile_pool(name="expp", bufs=34) as expp, \
             tc.tile_pool(name="attnp", bufs=2) as attnp, \
             tc.tile_pool(name="atp", bufs=4) as atp, \
             tc.tile_pool(name="denp", bufs=2) as denp, \
             tc.tile_pool(name="ytp", bufs=5) as ytp, \
             tc.tile_pool(name="mm_ps", bufs=4, space="PSUM") as mm_ps, \
             tc.tile_pool(name="scores", bufs=2, space="PSUM") as scores_p:
            W_sb = store.tile([128, NKT, 3 * G * HD], BF)
            Wo_sb = store.tile([128, NP, D], BF)
            kT_sb = store.tile([128, NP, L], BF)
            v_sb = store.tile([128, NKJ, G, HD + 1], BF)
            mi_sb = store.tile([128, 256], BF)
            id_sb = mi_sb[:, 0:128]
            masks_sb = mi_sb[:, 128:256]

            nc.vector.memset(v_sb[:, :, :, HD:HD + 1], 1.0)

            xT_r = xT.rearrange("kt p l -> p kt l")
            W_r = W_in.rearrange("kt p c -> p kt c")
            yT_r = yT.rearrange("(m p) l -> p m l", p=128)

            # DMA order = first-use order: xt0, W[q m01], W[k m01],
            # ident, xt1, masks, W[q m23], W[k m23], W[v], Wo.  Startup
            # loads go on the scalar queue (ACT idle then); mid-kernel x
            # chunks and yt outputs go on the SP queue so the ACT
            # sequencer never stalls on DMA config mid-exp-stream.
            xt_pre = [xtp.tile([128, NKT, CH], BF, name=f"xt{c}", tag="xt")
                      for c in range(2)]

            def wdma(g):
                nc.scalar.dma_start(out=W_sb[:, :, g * WG:(g + 1) * WG],
                                    in_=W_r[:, :, g * WG:(g + 1) * WG])

            nc.sync.dma_start(out=xt_pre[0][:], in_=xT_r[:, :, 0:CH])
            wdma(0)
            nc.gpsimd.dma_start(out=mi_sb[:], in_=masks[:])
            wdma(2)
            nc.sync.dma_start(out=xt_pre[1][:],
                               in_=xT_r[:, :, CH:2 * CH])
            wdma(1)
            wdma(3)
            wdma(4)
            wdma(5)
            nc.scalar.dma_start(
                out=Wo_sb[:], in_=W_out_s.rearrange("ct p d -> p ct d"))

            # p-state warmup: keep the PE continuously busy from t~0 so
            # it reaches full clock before the first real matmul's inputs
            # arrive (the first ~5us are DMA-bound anyway)
            warm = store.tile([128, 128], BF)
            nc.vector.memset(warm[:], 0.25)
            wps = mm_ps.tile([128, 128], F32, tag="ps", name="wps")
            for _ in range(44):
                nc.tensor.matmul(wps[:], warm[:], warm[:], start=True,
                                 stop=True)

            xts = {}

            def get_xt(c):
                if c not in xts:
                    if c < 2:
                        xts[c] = xt_pre[c]
                    else:
                        xt = xtp.tile([128, NKT, CH], BF, name=f"xt{c}",
                                      tag="xt")
                        nc.sync.dma_start(out=xt[:],
                                          in_=xT_r[:, :, c * CH:(c + 1) * CH])
                        xts[c] = xt
                return xts[c]

            KOFF = G * HD
            VOFF = 2 * G * HD

            def qk_m(c, m, qT_blk):
                """q and k projections for one m (head-pair) tile of one
                x chunk — the minimal unit on the first block's critical
                path."""
                xt = get_xt(c)
                half = (c % 2) * CH
                for off, out_ap in (
                        (m * 128, qT_blk[:, m, half:half + CH]),
                        (KOFF + m * 128, kT_sb[:, m, c * CH:c * CH + CH])):
                    ps = mm_ps.tile([128, CH], F32, tag="ps", name="psqk")
                    for kt in range(NKT):
                        nc.tensor.matmul(
                            ps[:], W_sb[:, kt, off:off + 128],
                            xt[:, kt, :], start=(kt == 0), stop=(kt == NKT - 1))
                    nc.vector.tensor_copy(out=out_ap, in_=ps[:])

            def v_proj(c):
                xt = get_xt(c)
                for sub in range(CH // KT):
                    ps = mm_ps.tile([128, G * HD], F32, tag="ps", name="psv")
                    for kt in range(NKT):
                        nc.tensor.matmul(
                            ps[:], xt[:, kt, sub * KT:(sub + 1) * KT],
                            W_sb[:, kt, VOFF:VOFF + G * HD],
                            start=(kt == 0), stop=(kt == NKT - 1))
                    nc.vector.tensor_copy(
                        out=v_sb[:, c * (CH // KT) + sub, :, 0:HD],
                        in_=ps[:].rearrange("p (h d) -> p h d", h=G))

            def q_parts(c, m, qT_blk):
                """Two ~426ns queue items sharing one open PSUM group, so
                a ready score matmul never waits behind a long fill item."""
                st = {}
                half = (c % 2) * CH

                def p0():
                    xt = get_xt(c)
                    st["ps"] = mm_ps.tile([128, CH], F32, tag="ps", name="psq")
                    for kt in range(4):
                        nc.tensor.matmul(
                            st["ps"][:], W_sb[:, kt, m * 128:(m + 1) * 128],
                            xt[:, kt, :], start=(kt == 0), stop=False)

                def p1():
                    xt = get_xt(c)
                    for kt in range(4, NKT):
                        nc.tensor.matmul(
                            st["ps"][:], W_sb[:, kt, m * 128:(m + 1) * 128],
                            xt[:, kt, :], start=False, stop=(kt == NKT - 1))
                    nc.vector.tensor_copy(out=qT_blk[:, m, half:half + CH],
                                          in_=st["ps"][:])
                return [(CH, p0), (CH, p1)]

            def k_parts(c, m):
                st = {}

                def p0():
                    xt = get_xt(c)
                    st["ps"] = mm_ps.tile([128, CH], F32, tag="ps", name="psk")
                    for kt in range(4):
                        nc.tensor.matmul(
                            st["ps"][:],
                            W_sb[:, kt, KOFF + m * 128:KOFF + (m + 1) * 128],
                            xt[:, kt, :], start=(kt == 0), stop=False)

                def p1():
                    xt = get_xt(c)
                    for kt in range(4, NKT):
                        nc.tensor.matmul(
                            st["ps"][:],
                            W_sb[:, kt, KOFF + m * 128:KOFF + (m + 1) * 128],
                            xt[:, kt, :], start=False, stop=(kt == NKT - 1))
                    nc.vector.tensor_copy(
                        out=kT_sb[:, m, c * CH:c * CH + CH], in_=st["ps"][:])
                return [(CH, p0), (CH, p1)]

            def v_parts(c, sub):
                st = {}

                def mk(kt0, kt1, first, last):
                    def p():
                        xt = get_xt(c)
                        if first:
                            st["ps"] = mm_ps.tile([128, G * HD], F32,
                                                  tag="ps", name="psv")
                        for kt in range(kt0, kt1):
                            nc.tensor.matmul(
                                st["ps"][:], xt[:, kt, sub * KT:(sub + 1) * KT],
                                W_sb[:, kt, VOFF:VOFF + G * HD],
                                start=(kt == 0), stop=(kt == NKT - 1))
                        if last:
                            nc.vector.tensor_copy(
                                out=v_sb[:, c * (CH // KT) + sub, :, 0:HD],
                                in_=st["ps"][:].rearrange("p (h d) -> p h d",
                                                          h=G))
                    return p
                return [(G * HD, mk(0, 2, True, False)),
                        (G * HD, mk(2, 4, False, False)),
                        (G * HD, mk(4, 6, False, False)),
                        (G * HD, mk(6, 8, False, True))]

            attn_nats = {}

            def av_seg(qb, pair, j, exs, st, t0, t1):
                """AV accumulation segment [t0, t1) for one qtile of a
                pair; the last segment normalizes out of PSUM."""
                hA, hB = 2 * pair, 2 * pair + 1
                jt = qb * (LC // KT) + j
                if t0 == 0:
                    if j == 0:
                        attn_nats[pair] = attnp.tile([128, 4, 128], BF,
                                                     name="anat")
                    st["avA"] = mm_ps.tile([128, LC], F32, tag="ps",
                                           name="avA")
                    st["avB"] = mm_ps.tile([128, LC], F32, tag="ps",
                                           name="avB")
                avA, avB = st["avA"], st["avB"]
                for t in range(t0, min(t1, jt + 1)):
                    nc.tensor.matmul(
                        avA[:, 0:HD + 1], exs[t][:, j * KT:(j + 1) * KT],
                        v_sb[:, t, hA, :], start=(t == 0), stop=(t == jt))
                    nc.tensor.matmul(
                        avB[:, 0:HD + 1],
                        exs[t][:, LC + j * KT:LC + (j + 1) * KT],
                        v_sb[:, t, hB, :], start=(t == 0), stop=(t == jt))
                if t1 < jt + 1:
                    return
                attn_nat = attn_nats[pair]
                den = denp.tile([128, 2], F32, name="den")
                for h2, av in ((0, avA), (1, avB)):
                    nc.vector.reciprocal(out=den[:, h2:h2 + 1],
                                         in_=av[:, HD:HD + 1])
                    nc.vector.tensor_scalar(
                        out=attn_nat[:, j, h2 * HD:(h2 + 1) * HD],
                        in0=av[:, 0:HD], scalar1=den[:, h2:h2 + 1],
                        scalar2=None, op0=mybir.AluOpType.mult)

            def av_group(qb, pair, j, exs):
                st = {}
                av_seg(qb, pair, j, exs, st, 0, 99)

            def transpose_j(pair, j, attnT):
                attn_nat = attn_nats[pair]
                tp = mm_ps.tile([128, KT], BF, tag="ps", name="tp")
                nc.tensor.matmul(tp[:], attn_nat[:, j, :], id_sb,
                                 is_transpose=True)
                nc.vector.tensor_copy(
                    out=attnT[:, pair, j * KT:(j + 1) * KT], in_=tp[:])

            def transposes(pair, attnT):
                for j in range(4):
                    transpose_j(pair, j, attnT)
                attn_nats.pop(pair)

            def outproj_p0(qb, m, attnT, st):
                st["ps"] = mm_ps.tile([128, LC], F32, tag="ps", name="psy")
                for ct in range(2):
                    nc.tensor.matmul(
                        st["ps"][:], Wo_sb[:, ct, m * 128:(m + 1) * 128],
                        attnT[:, ct, :], start=(ct == 0), stop=False)

            def outproj_p1(qb, m, attnT, st):
                for ct in range(2, NP):
                    nc.tensor.matmul(
                        st["ps"][:], Wo_sb[:, ct, m * 128:(m + 1) * 128],
                        attnT[:, ct, :], start=False, stop=(ct == NP - 1))
                yt = ytp.tile([128, LC], F32, name="yt", tag="yt")
                nc.vector.tensor_copy(out=yt[:], in_=st["ps"][:])
                nc.sync.dma_start(out=yT_r[:, m, qb * LC:qb * LC + LC],
                                  in_=yt[:])

            yt_last = {}

            def outproj_mj(m, j, attnT, yts):
                ps = mm_ps.tile([128, KT], F32, tag="ps", name="psj")
                for ct in range(NP):
                    nc.tensor.matmul(
                        ps[:], Wo_sb[:, ct, m * 128:(m + 1) * 128],
                        attnT[:, ct, j * KT:(j + 1) * KT],
                        start=(ct == 0), stop=(ct == NP - 1))
                nc.vector.tensor_copy(out=yts[m][:, j * KT:(j + 1) * KT],
                                      in_=ps[:])

            def outproj_m(qb, m, attnT):
                ps = mm_ps.tile([128, LC], F32, tag="ps", name="psy")
                for ct in range(NP):
                    nc.tensor.matmul(
                        ps[:], Wo_sb[:, ct, m * 128:(m + 1) * 128],
                        attnT[:, ct, :], start=(ct == 0), stop=(ct == NP - 1))
                yt = ytp.tile([128, LC], F32, name="yt", tag="yt")
                nc.vector.tensor_copy(out=yt[:], in_=ps[:])
                nc.sync.dma_start(out=yT_r[:, m, qb * LC:qb * LC + LC],
                                  in_=yt[:])

            # Global fill queue: every PE task that is not on the ACT
            # critical path (scores+exp) is queued with a row-cost and
            # drained into the t-loops at the rate the exp stream frees PE
            # cycles (exp runs at 0.833 ns/col on ACT vs 0.4167 ns/row on
            # PE, so each exp column buys about one spare PE row beyond
            # the score matmuls).  Unspent inventory floats forward into
            # the ACT-heavy late blocks; dependency-forced items are
            # drained explicitly at pair/block boundaries.
            fillq = []
            opq = []   # out-projections: lowest priority, float late
            cur_pos = [0]

            def qfill(rows, fn, cls="", min_qb=-1):
                fns = fn if isinstance(fn, list) else [fn]
                fillq.append([rows, fns, cls, min_qb])

            def _pick(q, cls):
                for i, (rows, fns, c, mq) in enumerate(q):
                    if cls is not None:
                        if c == cls:
                            return i
                        continue
                    if mq <= cur_pos[0]:
                        return i
                return None

            def drain(budget=None, cls=None):
                while True:
                    if budget is not None and budget <= 0:
                        break
                    i = _pick(fillq, cls)
                    if i is not None:
                        rows, fns, c, mq = fillq.pop(i)
                    elif cls is None:
                        j = _pick(opq, None)
                        if j is None:
                            break
                        rows, fns, c, mq = opq.pop(j)
                    else:
                        break
                    for fn in fns:
                        fn()
                    if budget is not None:
                        budget -= rows

            def queue_proj(c, qT_blk):
                for m in range(NP):
                    parts = q_parts(c, m, qT_blk)
                    qfill(sum(r for r, _ in parts), [f for _, f in parts],
                          f"q{c}", (c // 2 - 1) * NP + 2)
                for m in range(NP):
                    parts = k_parts(c, m)
                    qfill(sum(r for r, _ in parts), [f for _, f in parts],
                          f"kv{c}", (c // 2) * NP)
                for sub in range(CH // KT):
                    parts = v_parts(c, sub)
                    qfill(sum(r for r, _ in parts), [f for _, f in parts],
                          f"kv{c}", (c // 2) * NP)

            def queue_av(qb, pair, exs, attnT):
                for j in range(4):
                    jt = qb * (LC // KT) + j
                    qfill(2 * (HD + 1) * (jt + 1),
                          lambda j=j: av_group(qb, pair, j, exs),
                          f"av{pair}")
                for j in range(4):
                    qfill(KT, lambda j=j: transpose_j(pair, j, attnT),
                          f"av{pair}")
                qfill(0, lambda: attn_nats.pop(pair), f"av{pair}")

            def attention(qb, qT_blk, attnT, qT_next):
                n_t = (qb + 1) * (LC // KT)
                last = qb == NLC - 1
                if qT_next is not None:
                    queue_proj(2 * qb + 2, qT_next)
                    qfill(0, lambda: None)
                    queue_proj(2 * qb + 3, qT_next)
                if qb == 0:
                    # pair-0 critical pieces first, then the rest of
                    # chunk 0 (ready as soon as xt0 lands — fills the
                    # PE while xt1 is still in flight), then chunk 1
                    qk_m(0, 0, qT_blk)
                    qk_m(1, 0, qT_blk)
                    for m in range(1, NP):
                        qk_m(0, m, qT_blk)
                    for m in range(1, NP):
                        qk_m(1, m, qT_blk)
                for pair in range(NP):
                    cur_pos[0] = qb * NP + pair
                    exs = []
                    pace_own_av = last and pair == NP - 1
                    for t in range(n_t):
                        diag = t >= qb * (LC // KT)
                        o = t - qb * (LC // KT) if diag else 0
                        z = o * KT
                        if diag and o == 0 and qb > 0:
                            drain(cls=f"kv{2 * qb}")
                        if diag and o == 2 and qb > 0:
                            drain(cls=f"kv{2 * qb + 1}")
                        sc = scores_p.tile([128, 2 * LC], F32, tag="sc")
                        nc.tensor.matmul(
                            sc[:, z:LC],
                            kT_sb[0:64, pair, t * KT:(t + 1) * KT],
                            qT_blk[0:64, pair, z:LC], start=True,
                            stop=not diag)
                        nc.tensor.matmul(
                            sc[:, LC + z:2 * LC],
                            kT_sb[64:128, pair, t * KT:(t + 1) * KT],
                            qT_blk[64:128, pair, z:LC], start=True,
                            stop=not diag)
                        if diag:
                            nc.tensor.matmul(sc[:, z:z + KT], id_sb,
                                             masks_sb,
                                             start=False, stop=True)
                            nc.tensor.matmul(sc[:, LC + z:LC + z + KT],
                                             id_sb, masks_sb,
                                             start=False, stop=True)
                        ex = expp.tile([128, 2 * LC], BF)
                        sc_v = sc[:].rearrange("p (h c) -> p h c", h=2)[:, :, z:LC]
                        ex_v = ex[:].rearrange("p (h c) -> p h c", h=2)[:, :, z:LC]
                        nc.scalar.activation(ex_v, sc_v, AF.Exp, scale=scale)
                        exs.append(ex)
                        if pace_own_av and diag:
                            av_group(qb, pair, o, exs)
                        spare = 2 * (LC - z) - (256 if diag else 0)
                        drain(budget=int(spare * 0.75))
                    # dependency-forced drains: the pair-before-last's AV
                    # must complete (exp-tile pool bound) ...
                    if pair >= 1:
                        drain(cls=f"av{pair - 1}")
                    if pace_own_av:
                        transposes(pair, attnT)
                    else:
                        queue_av(qb, pair, exs, attnT)
                # ... and the next block's q projections before its
                # scores (k/v only feed its diagonal tiles: drained there)
                if qT_next is not None:
                    drain(cls=f"q{2 * qb + 2}")
                    drain(cls=f"q{2 * qb + 3}")
                if not last:
                    drain(cls=f"av{NP - 1}")

            attnTs = {}
            qTs = {0: qtp.tile([128, NP, LC], BF, name="qT0", tag="qT")}
            for qb in range(NLC):
                cur_pos[0] = qb * NP
                if qb + 1 < NLC:
                    qTs[qb + 1] = qtp.tile([128, NP, LC], BF,
                                           name=f"qT{qb + 1}", tag="qT")
                attnTs[qb] = atp.tile([128, NP, LC], BF, name=f"aT{qb}",
                                      tag="aT")
                if qb == 0:
                    for c in range(2):
                        for sub in range(2):
                            parts = v_parts(c, sub)
                            qfill(sum(r for r, _ in parts),
                                  [f for _, f in parts], f"kv{c}")
                attention(qb, qTs[qb], attnTs[qb], qTs.get(qb + 1))
                if qb < NLC - 1:
                    for m in range(D // 128):
                        opq.append([NP * LC,
                                    [lambda m=m, qb=qb:
                                     outproj_m(qb, m, attnTs[qb])], "op",
                                    (NLC - 1) * NP])
            cur_pos[0] = NLC * NP
            drain()
            for m in range(D // 128):
                outproj_m(NLC - 1, m, attnTs[NLC - 1])
    nc.compile()
    return nc


def _make_masks():
    import ml_dtypes
    r = np.arange(128)[:, None]
    c = np.arange(128)[None, :]
    return np.where(c >= r, 0.0, -1e6).astype(ml_dtypes.bfloat16)


def _make_ident():
    import ml_dtypes
    return np.eye(128, dtype=ml_dtypes.bfloat16)


def kernel(x, W_qkv, b_qkv, W_out, b_out, _trace=False, _trace_kwargs=None):
    import ml_dtypes
    bf16 = ml_dtypes.bfloat16
    x = np.ascontiguousarray(x, dtype=np.float32)
    W_qkv = np.asarray(W_qkv, dtype=np.float32)
    b_qkv = np.asarray(b_qkv, dtype=np.float32)
    W_out = np.asarray(W_out, dtype=np.float32)
    b_out = np.asarray(b_out, dtype=np.float32)
    assert np.all(b_qkv == 0.0), "nonzero b_qkv not supported by this kernel"

    if "nc" not in _cache:
        _cache["nc"] = _build()
    nc = _cache["nc"]

    masks = _make_masks()
    ident = _make_ident()
    Wq, Wk, Wv = W_qkv[:, 0:D], W_qkv[:, D:2 * D], W_qkv[:, 2 * D:3 * D]

    in_maps = []
    for c in range(8):
        b, g = divmod(c, 2)
        cols = slice(g * G * HD, (g + 1) * G * HD)
        W_in = np.concatenate([Wq[:, cols], Wk[:, cols], Wv[:, cols]], axis=1)
        in_maps.append({
            "xT": np.ascontiguousarray(x[b].T).astype(bf16).reshape(
                NKT, 128, L),
            "W_in": W_in.astype(bf16).reshape(NKT, 128, 3 * G * HD),
            "W_out_s": W_out[cols, :].astype(bf16).reshape(NP, 128, D),
            "masks": np.concatenate([ident, masks], axis=1),
        })

    kw = {}
    if _trace:
        kw["trace"] = True
        kw.update(_trace_kwargs or {})
    res = run_bass_kernel_spmd(nc, in_maps, list(range(8)), **kw)

    out = np.empty((B, L, D), dtype=np.float32)
    for b in range(B):
        yT = res.results[2 * b]["yT"] + res.results[2 * b + 1]["yT"]
        out[b] = yT.T + b_out
    if _trace:
        _cache["last_result"] = res
    return out
